# revision 1
# baseline (speedup 1.0000x reference)
"""Causal self-attention with RoPE for trn2, 8-core SPMD.

Problem (hardcoded): B=2, T=2048, C=1024, 16 heads, head_dim=64, fp32 io.
  qkv = x @ w_attn.T; q,k roped; causal softmax(q k^T/8) v; y @ w_proj.T

Sharding: core c -> (batch b = c//4, head-group g = c%4) — 4 heads per core.
Each core computes its group's partial output projection; host sums the 4
group partials per batch.

Device layout (per core):
  xT   [C, T]  f32   — x[b] transposed on host (feeds matmul contraction dim)
  wqkT [C, 512] bf16 — [Wq_g | Wk_g] transposed (cols: 4 heads x 64 q, then k)
  wvT  [C, 260] bf16 — Wv_g transposed, padded: per head 64 cols + 1 zero col
                       (the zero col becomes the "ones" column for sum-exp)
  wpT  [256, C] bf16 — w_proj[:, group cols] transposed
  cosT/sinT [128, T] bf16 — RoPE tables transposed, 2-head stacked; sinT rows
                       0:32/64:96 pre-negated so rope = q*cos + swap32(q)*sin
  masks [4, 128, 1024] bf16 — causal 0/1 masks for the 4 diagonal offsets
  out  [T, C]  f32   — partial output (host sums the 4 group partials)

Pipeline, interleaved per 512-token block so the DVE-heavy projection work
overlaps the ACT-heavy attention work:
  per tb: QK^T projection (w stationary -> transposed layout; RoPE on DVE
  with partition-shifted strip copies) + V projection (x stationary ->
  natural layout + ones column) -> flash attention for qb=tb per head pair:
  S^T tiles on PE (row-packed pairs), exp on ACT straight from PSUM, causal
  mask on DVE (multiply by 0/1 mask), [V|1] matmul (M=65) accumulates y^T
  and sum-exp together in PSUM; 1/sum-exp via exp(-ln(s)) on ACT; broadcast
  on gpsimd; final projection back to natural layout.
"""

from contextlib import ExitStack

import numpy as np
import ml_dtypes

import concourse.bass as bass
import concourse.tile as tile
from concourse import bacc, mybir
from concourse.bass_utils import run_bass_kernel_spmd

B, T, C = 2, 2048, 1024
NH, HD = 16, 64
HG = 4              # heads per group (per core)
GD = HG * HD        # 256
NCC = C // 128      # 8 contraction chunks
F32 = mybir.dt.float32
BF16 = mybir.dt.bfloat16
BF = ml_dtypes.bfloat16

QB = 512            # query block size
KT = 128            # key tile size


def build_kernel(t=T):
    nc = bacc.Bacc("TRN2", target_bir_lowering=False, debug=False)
    xT = nc.dram_tensor("xT", [C, t], F32, kind="ExternalInput").ap()
    wqkT = nc.dram_tensor("wqkT", [C, 2 * GD], BF16, kind="ExternalInput").ap()
    wvT = nc.dram_tensor("wvT", [C, HG * (HD + 1)], BF16,
                         kind="ExternalInput").ap()
    wpT = nc.dram_tensor("wpT", [GD, C], BF16, kind="ExternalInput").ap()
    cosT = nc.dram_tensor("cosT", [128, t], BF16, kind="ExternalInput").ap()
    sinT = nc.dram_tensor("sinT", [128, t], BF16, kind="ExternalInput").ap()
    masks = nc.dram_tensor("masks", [QB // KT, 128, 2 * QB], BF16,
                           kind="ExternalInput").ap()
    out = nc.dram_tensor("out", [t, C], F32, kind="ExternalOutput").ap()

    with tile.TileContext(nc) as tc:
        _attn_body(tc, out, xT, wqkT, wvT, wpT, cosT, sinT, masks, t)
    nc.compile()
    return nc


def _attn_body(tc, out, xT, wqkT, wvT, wpT, cosT, sinT, masks, t):
    ctx = ExitStack()
    nc = tc.nc
    ntt = t // 128          # t tiles (and k tiles)
    nqb = t // QB           # query blocks (== t blocks)
    Exp = mybir.ActivationFunctionType.Exp
    Log = mybir.ActivationFunctionType.Ln

    consts = ctx.enter_context(tc.tile_pool(name="consts", bufs=1))
    resident = ctx.enter_context(tc.tile_pool(name="resident", bufs=1))
    xstage = ctx.enter_context(tc.tile_pool(name="xstage", bufs=2))
    ropet = ctx.enter_context(tc.tile_pool(name="ropet", bufs=3))
    exps = ctx.enter_context(tc.tile_pool(name="exps", bufs=10))
    small = ctx.enter_context(tc.tile_pool(name="small", bufs=2))
    outsb = ctx.enter_context(tc.tile_pool(name="outsb", bufs=4))
    psA = ctx.enter_context(tc.tile_pool(name="psA", bufs=2, space="PSUM"))
    psS = ctx.enter_context(tc.tile_pool(name="psS", bufs=2, space="PSUM"))
    psY = ctx.enter_context(tc.tile_pool(name="psY", bufs=2, space="PSUM"))

    # ---- constants in ----
    cos_sb = consts.tile([128, t], BF16)
    nc.sync.dma_start(cos_sb[:], cosT[:])
    sin_sb = consts.tile([128, t], BF16)
    nc.sync.dma_start(sin_sb[:], sinT[:])
    wqk_sb = consts.tile([128, NCC, 2 * GD], BF16)
    nc.sync.dma_start(wqk_sb[:], wqkT.rearrange("(cc p) j -> p cc j", p=128))
    wv_sb = consts.tile([128, NCC, HG * (HD + 1)], BF16)
    nc.sync.dma_start(wv_sb[:], wvT.rearrange("(cc p) j -> p cc j", p=128))
    wp_sb = consts.tile([128, 2, C], BF16)
    nc.sync.dma_start(wp_sb[:], wpT.rearrange("(jc p) c -> p jc c", p=128))
    mask_sb = consts.tile([128, QB // KT, 2 * QB], BF16)
    nc.sync.dma_start(mask_sb[:], masks.rearrange("d p q -> p d q"))

    x_sb = resident.tile([128, NCC, t], BF16, tag="x")
    qk = resident.tile([128, 4, t], BF16, tag="qk")
    v_sb = resident.tile([128, ntt * HG, HD + 1], BF16, tag="v")
    ynorm = resident.tile([128, 2, t], BF16, tag="ynorm")

    def load_x(cc):
        xs = xstage.tile([128, t], F32, tag="xs")
        nc.sync.dma_start(xs[:], xT[cc * 128:(cc + 1) * 128, :])
        nc.vector.tensor_copy(x_sb[:, cc, :], xs[:])

    def qk_proj(jt, tb):
        # qk chunks: 0 = q heads(0,1), 1 = q heads(2,3), 2 = k(0,1), 3 = k(2,3)
        # generator: yields every 2 contraction chunks (2 matmuls) so the
        # weaver can interleave without blocking the PE FIFO for long
        tsl = bass.ts(tb, QB)
        ps = psA.tile([128, QB], F32, tag="psA")
        for cc in range(NCC):
            nc.tensor.matmul(
                ps[:], wqk_sb[:, cc, bass.ts(jt, 128)], x_sb[:, cc, tsl],
                start=(cc == 0), stop=(cc == NCC - 1))
            if cc % 2 == 1:
                yield
        raw = ropet.tile([128, QB], BF16, tag="raw")
        nc.vector.tensor_copy(raw[:], ps[:])
        rot = ropet.tile([128, QB], BF16, tag="rot")
        for s in range(4):
            nc.vector.tensor_copy(rot[s * 32:(s + 1) * 32, :],
                                  raw[(s ^ 1) * 32:((s ^ 1) + 1) * 32, :])
        cosp = ropet.tile([128, QB], BF16, tag="cosp")
        nc.vector.tensor_mul(cosp[:], raw[:], cos_sb[:, tsl])
        sinp = ropet.tile([128, QB], BF16, tag="sinp")
        nc.vector.tensor_mul(sinp[:], rot[:], sin_sb[:, tsl])
        nc.vector.tensor_add(qk[:, jt, tsl], cosp[:], sinp[:])
        yield

    def v_proj(tt):
        # v layout [128, ntt*HG, 65]: (t-tile, local head): 64 cols + 1 ones
        ps = psA.tile([128, HG * (HD + 1)], F32, tag="psA")
        for cc in range(NCC):
            nc.tensor.matmul(
                ps[:], x_sb[:, cc, bass.ts(tt, 128)], wv_sb[:, cc, :],
                start=(cc == 0), stop=(cc == NCC - 1))
            if cc % 2 == 1:
                yield
        nc.vector.tensor_copy(
            v_sb[:, tt * HG:(tt + 1) * HG, :],
            ps.rearrange("p (h d) -> p h d", d=HD + 1))
        nc.vector.memset(v_sb[:, tt * HG:(tt + 1) * HG, HD], 1.0)
        yield

    def attention_steps(qb, p):
        """Generator yielding one kt-step at a time (software-pipelined:
        S^T/exp for kt runs one step ahead of the V matmuls)."""
        qsl = bass.ts(qb, QB)
        nkt = (qb + 1) * (QB // KT)
        qc = qk[:, p, :]
        kc = qk[:, 2 + p, :]
        ya = psY.tile([HD + 1, QB], F32, tag="psY")
        yb = psY.tile([HD + 1, QB], F32, tag="psY")
        ets = {}
        for kt in range(nkt + 1):
            # V matmuls for the previous kt go first: they are ready (their
            # exp finished a step ago) while S(kt) may still wait on a PSUM
            # slot — keep the PE FIFO unblocked
            if kt >= 1:
                kv = kt - 1
                et = ets.pop(kv)
                first, last = (kv == 0), (kv == nkt - 1)
                nc.tensor.matmul(ya[:], v_sb[:, kv * HG + 2 * p, :],
                                 et[:, 0:QB], start=first, stop=last)
                nc.tensor.matmul(yb[:], v_sb[:, kv * HG + 2 * p + 1, :],
                                 et[:, QB:2 * QB], start=first, stop=last)
                yield
            if kt < nkt:
                ksl = bass.ts(kt, KT)
                pss = psS.tile([128, 2 * QB], F32, tag="psS")
                nc.tensor.matmul(pss[:, 0:QB], kc[0:64, ksl], qc[0:64, qsl],
                                 start=True, stop=True, tile_position=(0, 0))
                nc.tensor.matmul(pss[:, QB:2 * QB], kc[64:128, ksl],
                                 qc[64:128, qsl],
                                 start=True, stop=True, tile_position=(64, 0))
                et = exps.tile([128, 2 * QB], BF16, tag="exps")
                nc.scalar.activation(et[:], pss[:], Exp, scale=0.125)
                d = kt - qb * (QB // KT)
                if d >= 0:  # diagonal tile: causal mask on DVE
                    nc.vector.tensor_mul(et[:], et[:], mask_sb[:, d, :])
                ets[kt] = et
            yield
        # 1/sumexp = exp(-ln(s)) on ACT (DVE reciprocal is an 8-cycle-per-
        # element iterative divide; gpsimd has no tensor-tensor ALU here)
        lab = small.tile([1, 2 * QB], F32, tag="lab")
        nc.scalar.activation(lab[:, 0:QB], ya[HD:HD + 1, :], Log)
        nc.scalar.activation(lab[:, QB:2 * QB], yb[HD:HD + 1, :], Log)
        rab = small.tile([1, 2 * QB], F32, tag="rab")
        nc.scalar.activation(rab[:], lab[:], Exp, scale=-1.0)
        for h01, yp in ((0, ya), (1, yb)):
            rb = small.tile([64, QB], F32, tag="rb")
            nc.gpsimd.partition_broadcast(rb[:],
                                          rab[:, h01 * QB:(h01 + 1) * QB])
            nc.vector.tensor_mul(ynorm[h01 * 64:(h01 + 1) * 64, p, qsl],
                                 yp[0:HD, :], rb[:])
        yield

    def out_proj(tt, cb):
        ps = psA.tile([128, QB], F32, tag="psA")
        for jc in range(2):
            nc.tensor.matmul(
                ps[:], ynorm[:, jc, bass.ts(tt, 128)],
                wp_sb[:, jc, bass.ts(cb, QB)],
                start=(jc == 0), stop=(jc == 1))
        ot = outsb.tile([128, QB], F32, tag="ot")
        nc.vector.tensor_copy(ot[:], ps[:])
        nc.sync.dma_start(
            out[tt * 128:(tt + 1) * 128, bass.ts(cb, QB)], ot[:])
        yield

    # ---- interleaved schedule ----
    # Projection for block tb feeds attention for qb=tb (causal attention
    # needs K/V only up to the diagonal).  Attention's kt-steps for block tb
    # are woven with small granules of the *next* block's projection and the
    # *previous* block's output projection, so the PE always has a little
    # independent matmul work queued while ACT chews through exp, without
    # long FIFO chains delaying the next S^T matmul.
    from collections import deque
    proj_fill = deque()   # next block's qk/v projection granules
    out_fill = deque()    # completed blocks' output-projection granules

    def drain(n):
        for _ in range(n):
            q = proj_fill if proj_fill else out_fill
            if not q:
                return
            g = q.popleft()
            try:
                next(g)
                q.append(g)
            except StopIteration:
                pass

    def force(q):
        while q:
            g = q.popleft()
            for _ in g:
                pass

    for cc in range(NCC):
        load_x(cc)
    for jt in range(4):
        for _ in qk_proj(jt, 0):
            pass
    for tt in range(4):
        for _ in v_proj(tt):
            pass
    for tb in range(nqb):
        force(proj_fill)  # attention(tb) needs block tb's projections traced
        if tb + 1 < nqb:
            for jt in range(4):
                proj_fill.append(qk_proj(jt, tb + 1))
            for tt in range((tb + 1) * 4, (tb + 1) * 4 + 4):
                proj_fill.append(v_proj(tt))
        for p in range(2):
            for _ in attention_steps(tb, p):
                drain(1)
        for tt in range(tb * 4, tb * 4 + 4):
            for cb in range(2):
                out_fill.append(out_proj(tt, cb))
    force(proj_fill)
    force(out_fill)
    ctx.close()


def host_inputs(x, w_attn, w_proj, t=T):
    """Build the 8 per-core input maps from full inputs."""
    xTs = [np.ascontiguousarray(x[b, :t].T).astype(np.float32)
           for b in range(B)]
    inv = 1.0 / (10000.0 ** (np.arange(0, HD, 2, dtype=np.float32) / HD))
    fr = np.outer(np.arange(t, dtype=np.float32), inv)     # [t, 32]
    emb = np.concatenate([fr, fr], 1)                      # [t, 64]
    cos = np.cos(emb).T.astype(np.float32)                 # [64, t]
    sin = np.sin(emb).T.astype(np.float32)
    sin_s = sin.copy()
    sin_s[:32] *= -1.0
    cosT2 = np.tile(cos, (2, 1)).astype(BF)
    sinT2 = np.tile(sin_s, (2, 1)).astype(BF)

    # causal masks for the 4 diagonal offsets: keep iff q >= k, i.e.
    # f >= p + d*KT for f in [0,QB), p in [0,KT), doubled over head pair
    f = np.arange(QB)[None, :]
    pp = np.arange(KT)[:, None]
    m = np.stack([(f >= pp + d * KT) for d in range(QB // KT)], 0)
    masks = np.concatenate([m, m], axis=2).astype(BF)      # [4, 128, 1024]

    in_maps = []
    for c in range(8):
        b, g = c // 4, c % 4
        wq = w_attn[g * GD:(g + 1) * GD]
        wk = w_attn[C + g * GD:C + (g + 1) * GD]
        wv = w_attn[2 * C + g * GD:2 * C + (g + 1) * GD]
        wqkT = np.ascontiguousarray(
            np.concatenate([wq, wk], 0).T).astype(BF)
        wvT = np.zeros((C, HG * (HD + 1)), BF)
        for h in range(HG):
            wvT[:, h * (HD + 1):h * (HD + 1) + HD] = \
                wv[h * HD:(h + 1) * HD].T.astype(BF)
        wpT = np.ascontiguousarray(
            w_proj[:, g * GD:(g + 1) * GD].T).astype(BF)
        in_maps.append({"xT": xTs[b], "wqkT": wqkT, "wvT": wvT,
                        "wpT": wpT, "cosT": cosT2, "sinT": sinT2,
                        "masks": masks})
    return in_maps


_cache = {}


def kernel(x, w_attn, w_proj):
    x = np.asarray(x, dtype=np.float32)
    w_attn = np.asarray(w_attn, dtype=np.float32)
    w_proj = np.asarray(w_proj, dtype=np.float32)
    if "nc" not in _cache:
        _cache["nc"] = build_kernel()
    nc = _cache["nc"]
    in_maps = host_inputs(x, w_attn, w_proj)
    res = run_bass_kernel_spmd(nc, in_maps, list(range(8)))
    out = np.zeros((B, T, C), dtype=np.float32)
    for c in range(8):
        out[c // 4] += res.results[c]["out"]
    return out



# revision 11
# speedup vs baseline: 1.3009x; 1.3009x over previous
"""Causal self-attention with RoPE for trn2, 8-core SPMD.

Problem (hardcoded): B=2, T=2048, C=1024, 16 heads, head_dim=64, fp32 io.
  qkv = x @ w_attn.T; q,k roped; causal softmax(q k^T/8) v; y @ w_proj.T

Sharding: core c -> (batch b = c//4, head-group g = c%4) — 4 heads per core.
Each core computes its group's partial output projection; host sums the 4
group partials per batch.

Device layout (per core):
  xT   [C, T]  bf16  — x[b] transposed + pre-cast on host
  wqkT [C, 512] bf16 — [Wq_g | Wk_g] transposed (cols: 4 heads x 64 q, then k)
  wvT  [C, 260] bf16 — Wv_g transposed, padded: per head 64 cols + 1 zero col
                       (the zero col becomes the "ones" column for sum-exp)
  wpT  [256, C] bf16 — w_proj[:, group cols] transposed
  cosT/sinT [128, T] bf16 — RoPE tables transposed, 2-head stacked; sinT rows
                       0:32/64:96 pre-negated so rope = q*cos + swap32(q)*sin
  mask [128, 128] bf16 — within-tile causal 0/1 mask (keep iff f >= p)
  ident [128, 128] bf16 — identity for PE transposes
  out  [T, C]  bf16  — partial output (host sums the 4 group partials in f32)

Attention per (query-block qb, head-pair p), flash-style but with the V
matmuls flipped: S^T tiles on PE (row-tiled concurrent pairs), exp on ACT
straight from PSUM (sliced at the causal diagonal), 0/1 mask only on the
[128,128] diagonal sub-tile; then per (head, 128-query sub-tile) the exp'd
S^T tile is the STATIONARY operand and [V|1] streams (N=65), accumulating
y in [query-partition, dim] PSUM layout with sum-exp in column 64.  That
layout makes the softmax denominator per-partition: normalize = DVE
reciprocal [128,few] + one scalar_tensor_tensor per tile, no activation
table swaps and no partition broadcasts.  A PE transpose (via identity)
puts normalized y back into [dim, token] layout for the output projection.
Fully-masked (query < key-tile) sub-tiles are skipped everywhere.
"""

from contextlib import ExitStack

import numpy as np
import ml_dtypes

import concourse.bass as bass
import concourse.tile as tile
from concourse import bacc, mybir
from concourse.bass_utils import run_bass_kernel_spmd

B, T, C = 2, 2048, 1024
NH, HD = 16, 64
HG = 4              # heads per group (per core)
GD = HG * HD        # 256
NCC = C // 128      # 8 contraction chunks
F32 = mybir.dt.float32
BF16 = mybir.dt.bfloat16
BF = ml_dtypes.bfloat16

QB = 512            # query block size
KT = 128            # key tile size
NQT = QB // KT      # query sub-tiles per block (4)


def build_kernel(t=T):
    nc = bacc.Bacc("TRN2", target_bir_lowering=False, debug=False)
    xT = nc.dram_tensor("xT", [C, t], BF16, kind="ExternalInput").ap()
    wqkT = nc.dram_tensor("wqkT", [C, 2 * GD], BF16, kind="ExternalInput").ap()
    wvT = nc.dram_tensor("wvT", [C, HG * (HD + 1)], BF16,
                         kind="ExternalInput").ap()
    wpT = nc.dram_tensor("wpT", [GD, C], BF16, kind="ExternalInput").ap()
    cosT = nc.dram_tensor("cosT", [128, t], BF16, kind="ExternalInput").ap()
    sinT = nc.dram_tensor("sinT", [128, t], BF16, kind="ExternalInput").ap()
    mask = nc.dram_tensor("mask", [128, KT], BF16, kind="ExternalInput").ap()
    ident = nc.dram_tensor("ident", [128, 128], BF16,
                           kind="ExternalInput").ap()
    out = nc.dram_tensor("out", [t, C], BF16, kind="ExternalOutput").ap()

    with tile.TileContext(nc) as tc:
        _attn_body(tc, out, xT, wqkT, wvT, wpT, cosT, sinT, mask, ident, t)
    nc.compile()
    return nc


def _attn_body(tc, out, xT, wqkT, wvT, wpT, cosT, sinT, mask, ident, t):
    ctx = ExitStack()
    nc = tc.nc
    ntt = t // 128          # t tiles (and k tiles)
    nqb = t // QB           # query blocks
    Exp = mybir.ActivationFunctionType.Exp
    Mul = mybir.AluOpType.mult
    Add = mybir.AluOpType.add

    consts = ctx.enter_context(tc.tile_pool(name="consts", bufs=1))
    resident = ctx.enter_context(tc.tile_pool(name="resident", bufs=1))
    ropet = ctx.enter_context(tc.tile_pool(name="ropet", bufs=3))
    exps = ctx.enter_context(tc.tile_pool(name="exps", bufs=34))
    small = ctx.enter_context(tc.tile_pool(name="small", bufs=2))
    yntp = ctx.enter_context(tc.tile_pool(name="yntp", bufs=2))
    outsb = ctx.enter_context(tc.tile_pool(name="outsb", bufs=4))
    psA = ctx.enter_context(tc.tile_pool(name="psA", bufs=2, space="PSUM"))
    psS = ctx.enter_context(tc.tile_pool(name="psS", bufs=2, space="PSUM"))
    psY = ctx.enter_context(tc.tile_pool(name="psY", bufs=2, space="PSUM"))

    # ---- constants in ----
    wqk_sb = consts.tile([128, NCC, 2 * GD], BF16)
    nc.sync.dma_start(wqk_sb[:], wqkT.rearrange("(cc p) j -> p cc j", p=128))
    cos_sb = consts.tile([128, t], BF16)
    nc.sync.dma_start(cos_sb[:], cosT[:])
    sin_sb = consts.tile([128, t], BF16)
    nc.sync.dma_start(sin_sb[:], sinT[:])
    wv_sb = consts.tile([128, NCC, HG * (HD + 1)], BF16)
    nc.sync.dma_start(wv_sb[:], wvT.rearrange("(cc p) j -> p cc j", p=128))
    mask_sb = consts.tile([128, KT], BF16)
    nc.sync.dma_start(mask_sb[:], mask[:])
    id_sb = consts.tile([128, 128], BF16)
    nc.sync.dma_start(id_sb[:], ident[:])
    wp_sb = consts.tile([128, 2, C], BF16)
    nc.sync.dma_start(wp_sb[:], wpT.rearrange("(jc p) c -> p jc c", p=128))

    zeros = consts.tile([128, HD], BF16)
    nc.vector.memset(zeros[:], 0.0)

    x_sb = resident.tile([128, NCC, t], BF16, tag="x")
    qk = resident.tile([128, 4, t], BF16, tag="qk")
    v_sb = resident.tile([128, ntt * HG, HD + 1], BF16, tag="v")
    ynorm = resident.tile([128, 2, t], BF16, tag="ynorm")

    def load_x(cc):
        nc.sync.dma_start(x_sb[:, cc, :], xT[cc * 128:(cc + 1) * 128, :])

    def qk_proj(jt, tb):
        # qk chunks: 0 = q heads(0,1), 1 = q heads(2,3), 2 = k(0,1), 3 = k(2,3)
        tsl = bass.ts(tb, QB)
        ps = psA.tile([128, QB], F32, tag="psA")
        for cc in range(NCC):
            nc.tensor.matmul(
                ps[:], wqk_sb[:, cc, bass.ts(jt, 128)], x_sb[:, cc, tsl],
                start=(cc == 0), stop=(cc == NCC - 1))
            if cc % 2 == 1:
                yield
        raw = ropet.tile([128, QB], BF16, tag="raw")
        nc.vector.tensor_copy(raw[:], ps[:])
        rot = ropet.tile([128, QB], BF16, tag="rot")
        for s in range(4):
            nc.vector.tensor_copy(rot[s * 32:(s + 1) * 32, :],
                                  raw[(s ^ 1) * 32:((s ^ 1) + 1) * 32, :])
        cosp = ropet.tile([128, QB], BF16, tag="cosp")
        nc.vector.tensor_mul(cosp[:], raw[:], cos_sb[:, tsl])
        sinp = ropet.tile([128, QB], BF16, tag="sinp")
        nc.vector.tensor_mul(sinp[:], rot[:], sin_sb[:, tsl])
        nc.vector.tensor_add(qk[:, jt, tsl], cosp[:], sinp[:])
        yield

    def v_proj(tt):
        # v layout [128, ntt*HG, 65]: (t-tile, local head): 64 cols + 1 ones
        ps = psA.tile([128, HG * (HD + 1)], F32, tag="psA")
        for cc in range(NCC):
            nc.tensor.matmul(
                ps[:], x_sb[:, cc, bass.ts(tt, 128)], wv_sb[:, cc, :],
                start=(cc == 0), stop=(cc == NCC - 1))
            if cc % 2 == 1:
                yield
        nc.vector.tensor_copy(
            v_sb[:, tt * HG:(tt + 1) * HG, :],
            ps.rearrange("p (h d) -> p h d", d=HD + 1))
        nc.vector.memset(v_sb[:, tt * HG:(tt + 1) * HG, HD], 1.0)
        yield

    etstore = {}

    def attn_sx(qb, p):
        """Phase 1: all S^T + exp tiles for the block (kept in SBUF)."""
        q0 = qb * QB
        nkt = (qb + 1) * NQT
        qc = qk[:, p, :]
        kc = qk[:, 2 + p, :]
        ets = etstore.setdefault((qb, p), {})
        for kt in range(nkt):
            ksl = bass.ts(kt, KT)
            d = kt - qb * NQT
            off = d * KT if d > 0 else 0
            pss = psS.tile([128, 2 * QB], F32, tag="psS")
            nc.tensor.matmul(pss[:, off:QB], kc[0:64, ksl],
                             qc[0:64, q0 + off:q0 + QB],
                             start=True, stop=True, tile_position=(0, 0))
            nc.tensor.matmul(pss[:, QB + off:2 * QB], kc[64:128, ksl],
                             qc[64:128, q0 + off:q0 + QB],
                             start=True, stop=True, tile_position=(64, 0))
            et = exps.tile([128, 2 * QB], BF16, tag="exps")
            if off:
                nc.scalar.activation(et[:, off:QB], pss[:, off:QB],
                                     Exp, scale=0.125)
                nc.scalar.activation(et[:, QB + off:2 * QB],
                                     pss[:, QB + off:2 * QB],
                                     Exp, scale=0.125)
            else:
                nc.scalar.activation(et[:], pss[:], Exp, scale=0.125)
            if d >= 0:  # diagonal tile: 0/1 mask on the [128,KT] triangle
                c0 = d * KT
                nc.vector.tensor_mul(et[:, c0:c0 + KT],
                                     et[:, c0:c0 + KT], mask_sb[:])
                nc.vector.tensor_mul(et[:, QB + c0:QB + c0 + KT],
                                     et[:, QB + c0:QB + c0 + KT],
                                     mask_sb[:])
            ets[kt] = et
            yield

    def attn_v(qb, p):
        """Phase 2: V matmuls.  Each (head, qtile) accumulation runs as one
        contiguous PSUM group — never interleaved within a bank."""
        q0 = qb * QB
        ets = etstore.pop((qb, p))
        yh = [psY.tile([128, NQT, HD + 1], F32, tag="psY", name=f"yh{h}")
              for h in range(2)]
        for qt in range(NQT):
            for h in range(2):
                last_kv = qb * NQT + qt
                for kv in range(last_kv + 1):
                    nc.tensor.matmul(
                        yh[h][:, qt, :],
                        ets[kv][:, h * QB + qt * KT:h * QB + (qt + 1) * KT],
                        v_sb[:, kv * HG + 2 * p + h, :],
                        start=(kv == 0), stop=(kv == last_kv))
                    if kv % 4 == 3:
                        yield
                yield
        # ---- epilogue: per-partition normalize + transpose back ----
        rcp = small.tile([128, 2 * NQT], F32, tag="rcp")
        for h in range(2):
            nc.vector.reciprocal(rcp[:, h * NQT:(h + 1) * NQT],
                                 yh[h][:, :, HD])
        yield
        ynt = yntp.tile([128, NQT, 128], BF16, tag="ynt")
        for h in range(2):
            for qt in range(NQT):
                nc.vector.scalar_tensor_tensor(
                    ynt[:, qt, h * HD:(h + 1) * HD],
                    yh[h][:, qt, 0:HD],
                    rcp[:, h * NQT + qt:h * NQT + qt + 1],
                    zeros[:], op0=Mul, op1=Add)
            yield
        for qt in range(NQT):
            tps = psY.tile([128, 128], BF16, tag="psY", name=f"tps{qt}")
            nc.tensor.transpose(tps[:], ynt[:, qt, :], id_sb[:])
            nc.vector.tensor_copy(
                ynorm[:, p, q0 + qt * KT:q0 + (qt + 1) * KT], tps[:])
            yield

    def out_proj(tt, cb):
        ps = psA.tile([128, QB], F32, tag="psA")
        for jc in range(2):
            nc.tensor.matmul(
                ps[:], ynorm[:, jc, bass.ts(tt, 128)],
                wp_sb[:, jc, bass.ts(cb, QB)],
                start=(jc == 0), stop=(jc == 1))
        ot = outsb.tile([128, QB], BF16, tag="ot")
        nc.vector.tensor_copy(ot[:], ps[:])
        nc.sync.dma_start(
            out[tt * 128:(tt + 1) * 128, bass.ts(cb, QB)], ot[:])
        yield

    # ---- interleaved schedule ----
    # Projection for block tb feeds attention for qb=tb (causal attention
    # needs K/V only up to the diagonal).  Attention's kt-steps for block tb
    # are woven with small granules of the *next* block's projection and the
    # *previous* block's output projection, so the PE always has a little
    # independent matmul work queued while ACT chews through exp, without
    # long FIFO chains delaying the next S^T matmul.
    from collections import deque
    proj_fill = deque()   # next block's qk/v projection granules
    out_fill = deque()    # completed blocks' output-projection granules

    def drain(n):
        for _ in range(n):
            q = proj_fill if proj_fill else out_fill
            if not q:
                return
            g = q.popleft()
            try:
                next(g)
                q.append(g)
            except StopIteration:
                pass

    def force(q):
        while q:
            g = q.popleft()
            for _ in g:
                pass

    for cc in range(NCC):
        load_x(cc)
    for jt in range(4):
        for _ in qk_proj(jt, 0):
            pass
    for tt in range(4):
        for _ in v_proj(tt):
            pass
    for tb in range(nqb):
        force(proj_fill)  # attention(tb) needs block tb's projections traced
        if tb + 1 < nqb:
            for jt in range(4):
                proj_fill.append(qk_proj(jt, tb + 1))
            for tt in range((tb + 1) * 4, (tb + 1) * 4 + 4):
                proj_fill.append(v_proj(tt))
        for _ in attn_sx(tb, 0):
            drain(1)
        sx1 = attn_sx(tb, 1)
        proj_fill.appendleft(sx1)  # weave p1's S^T/exp into p0's V phase
        for _ in attn_v(tb, 0):
            drain(1)
        if sx1 in proj_fill:
            proj_fill.remove(sx1)
            for _ in sx1:
                drain(1)
        for _ in attn_v(tb, 1):
            drain(1)
        for tt in range(tb * 4, tb * 4 + 4):
            for cb in range(2):
                out_fill.append(out_proj(tt, cb))
    force(proj_fill)
    force(out_fill)
    ctx.close()


def host_inputs(x, w_attn, w_proj, t=T):
    """Build the 8 per-core input maps from full inputs."""
    xTs = [np.ascontiguousarray(x[b, :t].T).astype(BF) for b in range(B)]
    inv = 1.0 / (10000.0 ** (np.arange(0, HD, 2, dtype=np.float32) / HD))
    fr = np.outer(np.arange(t, dtype=np.float32), inv)     # [t, 32]
    emb = np.concatenate([fr, fr], 1)                      # [t, 64]
    cos = np.cos(emb).T.astype(np.float32)                 # [64, t]
    sin = np.sin(emb).T.astype(np.float32)
    sin_s = sin.copy()
    sin_s[:32] *= -1.0
    cosT2 = np.tile(cos, (2, 1)).astype(BF)
    sinT2 = np.tile(sin_s, (2, 1)).astype(BF)

    # within-tile causal mask: keep iff f >= p (f = query col, p = key row)
    f = np.arange(KT)[None, :]
    pp = np.arange(128)[:, None]
    mask = (f >= pp).astype(BF)                            # [128, KT]
    ident = np.eye(128, dtype=BF)

    in_maps = []
    for c in range(8):
        b, g = c // 4, c % 4
        wq = w_attn[g * GD:(g + 1) * GD]
        wk = w_attn[C + g * GD:C + (g + 1) * GD]
        wv = w_attn[2 * C + g * GD:2 * C + (g + 1) * GD]
        wqkT = np.ascontiguousarray(
            np.concatenate([wq, wk], 0).T).astype(BF)
        wvT = np.zeros((C, HG * (HD + 1)), BF)
        for h in range(HG):
            wvT[:, h * (HD + 1):h * (HD + 1) + HD] = \
                wv[h * HD:(h + 1) * HD].T.astype(BF)
        wpT = np.ascontiguousarray(
            w_proj[:, g * GD:(g + 1) * GD].T).astype(BF)
        in_maps.append({"xT": xTs[b], "wqkT": wqkT, "wvT": wvT,
                        "wpT": wpT, "cosT": cosT2, "sinT": sinT2,
                        "mask": mask, "ident": ident})
    return in_maps


_cache = {}


def kernel(x, w_attn, w_proj):
    x = np.asarray(x, dtype=np.float32)
    w_attn = np.asarray(w_attn, dtype=np.float32)
    w_proj = np.asarray(w_proj, dtype=np.float32)
    if "nc" not in _cache:
        _cache["nc"] = build_kernel()
    nc = _cache["nc"]
    in_maps = host_inputs(x, w_attn, w_proj)
    res = run_bass_kernel_spmd(nc, in_maps, list(range(8)))
    out = np.zeros((B, T, C), dtype=np.float32)
    for c in range(8):
        out[c // 4] += res.results[c]["out"].astype(np.float32)
    return out


# revision 12
# speedup vs baseline: 1.3508x; 1.0383x over previous
"""Causal self-attention with RoPE for trn2, 8-core SPMD.

Problem (hardcoded): B=2, T=2048, C=1024, 16 heads, head_dim=64, fp32 io.
  qkv = x @ w_attn.T; q,k roped; causal softmax(q k^T/8) v; y @ w_proj.T

Sharding: core c -> (batch b = c//4, head-group g = c%4) — 4 heads per core.
Each core computes its group's partial output projection; host sums the 4
group partials per batch.

Device layout (per core):
  xT   [C, T]  bf16  — x[b] transposed + pre-cast on host
  wqkT [C, 512] bf16 — [Wq_g | Wk_g] transposed (cols: 4 heads x 64 q, then k)
  wvT  [C, 260] bf16 — Wv_g transposed, padded: per head 64 cols + 1 zero col
                       (the zero col becomes the "ones" column for sum-exp)
  wpT  [256, C] bf16 — w_proj[:, group cols] transposed
  cosT/sinT [128, T] bf16 — RoPE tables transposed, 2-head stacked; sinT rows
                       0:32/64:96 pre-negated so rope = q*cos + swap32(q)*sin
  mask [128, 128] bf16 — within-tile causal 0/1 mask (keep iff f >= p)
  ident [128, 128] bf16 — identity for PE transposes
  out  [T, C]  bf16  — partial output (host sums the 4 group partials in f32)

Attention per (query-block qb, head-pair p), flash-style but with the V
matmuls flipped: S^T tiles on PE (row-tiled concurrent pairs), exp on ACT
straight from PSUM (sliced at the causal diagonal), 0/1 mask only on the
[128,128] diagonal sub-tile; then per (head, 128-query sub-tile) the exp'd
S^T tile is the STATIONARY operand and [V|1] streams (N=65), accumulating
y in [query-partition, dim] PSUM layout with sum-exp in column 64.  That
layout makes the softmax denominator per-partition: normalize = DVE
reciprocal [128,few] + one scalar_tensor_tensor per tile, no activation
table swaps and no partition broadcasts.  A PE transpose (via identity)
puts normalized y back into [dim, token] layout for the output projection.
Fully-masked (query < key-tile) sub-tiles are skipped everywhere.
"""

from contextlib import ExitStack

import numpy as np
import ml_dtypes

import concourse.bass as bass
import concourse.tile as tile
from concourse import bacc, mybir
from concourse.bass_utils import run_bass_kernel_spmd

B, T, C = 2, 2048, 1024
NH, HD = 16, 64
HG = 4              # heads per group (per core)
GD = HG * HD        # 256
NCC = C // 128      # 8 contraction chunks
F32 = mybir.dt.float32
BF16 = mybir.dt.bfloat16
BF = ml_dtypes.bfloat16

QB = 512            # query block size
KT = 128            # key tile size
NQT = QB // KT      # query sub-tiles per block (4)


def build_kernel(t=T):
    nc = bacc.Bacc("TRN2", target_bir_lowering=False, debug=False)
    xT = nc.dram_tensor("xT", [C, t], BF16, kind="ExternalInput").ap()
    wqkT = nc.dram_tensor("wqkT", [C, 2 * GD], BF16, kind="ExternalInput").ap()
    wvT = nc.dram_tensor("wvT", [C, HG * (HD + 1)], BF16,
                         kind="ExternalInput").ap()
    wpT = nc.dram_tensor("wpT", [GD, C], BF16, kind="ExternalInput").ap()
    cosT = nc.dram_tensor("cosT", [128, t], BF16, kind="ExternalInput").ap()
    sinT = nc.dram_tensor("sinT", [128, t], BF16, kind="ExternalInput").ap()
    mask = nc.dram_tensor("mask", [128, KT], BF16, kind="ExternalInput").ap()
    ident = nc.dram_tensor("ident", [128, 128], BF16,
                           kind="ExternalInput").ap()
    out = nc.dram_tensor("out", [t, C], BF16, kind="ExternalOutput").ap()

    with tile.TileContext(nc) as tc:
        _attn_body(tc, out, xT, wqkT, wvT, wpT, cosT, sinT, mask, ident, t)
    nc.compile()
    return nc


def _attn_body(tc, out, xT, wqkT, wvT, wpT, cosT, sinT, mask, ident, t):
    ctx = ExitStack()
    nc = tc.nc
    ntt = t // 128          # t tiles (and k tiles)
    nqb = t // QB           # query blocks
    Exp = mybir.ActivationFunctionType.Exp
    Mul = mybir.AluOpType.mult
    Add = mybir.AluOpType.add

    consts = ctx.enter_context(tc.tile_pool(name="consts", bufs=1))
    resident = ctx.enter_context(tc.tile_pool(name="resident", bufs=1))
    ropet = ctx.enter_context(tc.tile_pool(name="ropet", bufs=3))
    exps = ctx.enter_context(tc.tile_pool(name="exps", bufs=34))
    small = ctx.enter_context(tc.tile_pool(name="small", bufs=2))
    yntp = ctx.enter_context(tc.tile_pool(name="yntp", bufs=2))
    outsb = ctx.enter_context(tc.tile_pool(name="outsb", bufs=4))
    psA = ctx.enter_context(tc.tile_pool(name="psA", bufs=2, space="PSUM"))
    psS = ctx.enter_context(tc.tile_pool(name="psS", bufs=2, space="PSUM"))
    psY = ctx.enter_context(tc.tile_pool(name="psY", bufs=2, space="PSUM"))

    # ---- constants in ----
    wqk_sb = consts.tile([128, NCC, 2 * GD], BF16)
    nc.sync.dma_start(wqk_sb[:], wqkT.rearrange("(cc p) j -> p cc j", p=128))
    cos_sb = consts.tile([128, t], BF16)
    nc.sync.dma_start(cos_sb[:], cosT[:])
    sin_sb = consts.tile([128, t], BF16)
    nc.sync.dma_start(sin_sb[:], sinT[:])
    wv_sb = consts.tile([128, NCC, HG * (HD + 1)], BF16)
    nc.sync.dma_start(wv_sb[:], wvT.rearrange("(cc p) j -> p cc j", p=128))
    mask_sb = consts.tile([128, KT], BF16)
    nc.sync.dma_start(mask_sb[:], mask[:])
    id_sb = consts.tile([128, 128], BF16)
    nc.sync.dma_start(id_sb[:], ident[:])
    wp_sb = consts.tile([128, 2, C], BF16)
    nc.sync.dma_start(wp_sb[:], wpT.rearrange("(jc p) c -> p jc c", p=128))

    zeros = consts.tile([128, HD], BF16)
    nc.vector.memset(zeros[:], 0.0)

    x_sb = resident.tile([128, NCC, t], BF16, tag="x")
    qk = resident.tile([128, 4, t], BF16, tag="qk")
    v_sb = resident.tile([128, ntt * HG, HD + 1], BF16, tag="v")
    ynorm = resident.tile([128, 2, t], BF16, tag="ynorm")

    def load_x(cc):
        nc.sync.dma_start(x_sb[:, cc, :], xT[cc * 128:(cc + 1) * 128, :])

    def qk_proj(jt, tb):
        # qk chunks: 0 = q heads(0,1), 1 = q heads(2,3), 2 = k(0,1), 3 = k(2,3)
        tsl = bass.ts(tb, QB)
        ps = psA.tile([128, QB], F32, tag="psA")
        for cc in range(NCC):
            nc.tensor.matmul(
                ps[:], wqk_sb[:, cc, bass.ts(jt, 128)], x_sb[:, cc, tsl],
                start=(cc == 0), stop=(cc == NCC - 1))
            if cc % 2 == 1:
                yield
        raw = ropet.tile([128, QB], BF16, tag="raw")
        nc.vector.tensor_copy(raw[:], ps[:])
        rot = ropet.tile([128, QB], BF16, tag="rot")
        for s in range(4):
            nc.vector.tensor_copy(rot[s * 32:(s + 1) * 32, :],
                                  raw[(s ^ 1) * 32:((s ^ 1) + 1) * 32, :])
        cosp = ropet.tile([128, QB], BF16, tag="cosp")
        nc.vector.tensor_mul(cosp[:], raw[:], cos_sb[:, tsl])
        sinp = ropet.tile([128, QB], BF16, tag="sinp")
        nc.vector.tensor_mul(sinp[:], rot[:], sin_sb[:, tsl])
        nc.vector.tensor_add(qk[:, jt, tsl], cosp[:], sinp[:])
        yield

    def v_proj(tt):
        # v layout [128, ntt*HG, 65]: (t-tile, local head): 64 cols + 1 ones
        ps = psA.tile([128, HG * (HD + 1)], F32, tag="psA")
        for cc in range(NCC):
            nc.tensor.matmul(
                ps[:], x_sb[:, cc, bass.ts(tt, 128)], wv_sb[:, cc, :],
                start=(cc == 0), stop=(cc == NCC - 1))
            if cc % 2 == 1:
                yield
        nc.vector.tensor_copy(
            v_sb[:, tt * HG:(tt + 1) * HG, :],
            ps.rearrange("p (h d) -> p h d", d=HD + 1))
        nc.vector.memset(v_sb[:, tt * HG:(tt + 1) * HG, HD], 1.0)
        yield

    etstore = {}

    def attn_sx(qb, p):
        """Phase 1: all S^T + exp tiles for the block (kept in SBUF)."""
        q0 = qb * QB
        nkt = (qb + 1) * NQT
        qc = qk[:, p, :]
        kc = qk[:, 2 + p, :]
        ets = etstore.setdefault((qb, p), {})
        for kt in range(nkt):
            ksl = bass.ts(kt, KT)
            d = kt - qb * NQT
            off = d * KT if d > 0 else 0
            pss = psS.tile([128, 2 * QB], F32, tag="psS")
            nc.tensor.matmul(pss[:, off:QB], kc[0:64, ksl],
                             qc[0:64, q0 + off:q0 + QB],
                             start=True, stop=True, tile_position=(0, 0))
            nc.tensor.matmul(pss[:, QB + off:2 * QB], kc[64:128, ksl],
                             qc[64:128, q0 + off:q0 + QB],
                             start=True, stop=True, tile_position=(64, 0))
            et = exps.tile([128, 2 * QB], BF16, tag="exps")
            if off:
                nc.scalar.activation(et[:, off:QB], pss[:, off:QB],
                                     Exp, scale=0.125)
                nc.scalar.activation(et[:, QB + off:2 * QB],
                                     pss[:, QB + off:2 * QB],
                                     Exp, scale=0.125)
            else:
                nc.scalar.activation(et[:], pss[:], Exp, scale=0.125)
            if d >= 0:  # diagonal tile: 0/1 mask on the [128,KT] triangle
                c0 = d * KT
                nc.vector.tensor_mul(et[:, c0:c0 + KT],
                                     et[:, c0:c0 + KT], mask_sb[:])
                nc.vector.tensor_mul(et[:, QB + c0:QB + c0 + KT],
                                     et[:, QB + c0:QB + c0 + KT],
                                     mask_sb[:])
            ets[kt] = et
            yield

    def attn_v(qb, p):
        """Phase 2: V matmuls.  Each (head, qtile) accumulation runs as one
        contiguous PSUM group — never interleaved within a bank."""
        q0 = qb * QB
        ets = etstore.pop((qb, p))
        yh = [psY.tile([128, NQT, HD + 1], F32, tag="psY", name=f"yh{h}")
              for h in range(2)]
        for qt in range(NQT):
            for h in range(2):
                last_kv = qb * NQT + qt
                for kv in range(last_kv + 1):
                    nc.tensor.matmul(
                        yh[h][:, qt, :],
                        ets[kv][:, h * QB + qt * KT:h * QB + (qt + 1) * KT],
                        v_sb[:, kv * HG + 2 * p + h, :],
                        start=(kv == 0), stop=(kv == last_kv))
                    if kv % 4 == 3:
                        yield
                yield
        # ---- epilogue: per-partition normalize + transpose back ----
        rcp = small.tile([128, 2 * NQT], F32, tag="rcp")
        for h in range(2):
            nc.vector.reciprocal(rcp[:, h * NQT:(h + 1) * NQT],
                                 yh[h][:, :, HD])
        yield
        ynt = yntp.tile([128, NQT, 128], BF16, tag="ynt")
        for h in range(2):
            for qt in range(NQT):
                nc.vector.scalar_tensor_tensor(
                    ynt[:, qt, h * HD:(h + 1) * HD],
                    yh[h][:, qt, 0:HD],
                    rcp[:, h * NQT + qt:h * NQT + qt + 1],
                    zeros[:], op0=Mul, op1=Add)
            yield
        for qt in range(NQT):
            tps = psY.tile([128, 128], BF16, tag="psY", name=f"tps{qt}")
            nc.tensor.transpose(tps[:], ynt[:, qt, :], id_sb[:])
            nc.vector.tensor_copy(
                ynorm[:, p, q0 + qt * KT:q0 + (qt + 1) * KT], tps[:])
            yield

    def out_proj(tt, cb):
        ps = psA.tile([128, QB], F32, tag="psA")
        for jc in range(2):
            nc.tensor.matmul(
                ps[:], ynorm[:, jc, bass.ts(tt, 128)],
                wp_sb[:, jc, bass.ts(cb, QB)],
                start=(jc == 0), stop=(jc == 1))
        ot = outsb.tile([128, QB], BF16, tag="ot")
        nc.vector.tensor_copy(ot[:], ps[:])
        nc.sync.dma_start(
            out[tt * 128:(tt + 1) * 128, bass.ts(cb, QB)], ot[:])
        yield

    # ---- interleaved schedule ----
    # Projection for block tb feeds attention for qb=tb (causal attention
    # needs K/V only up to the diagonal).  Attention's kt-steps for block tb
    # are woven with small granules of the *next* block's projection and the
    # *previous* block's output projection, so the PE always has a little
    # independent matmul work queued while ACT chews through exp, without
    # long FIFO chains delaying the next S^T matmul.
    from collections import deque
    proj_fill = deque()   # next block's qk/v projection granules
    out_fill = deque()    # completed blocks' output-projection granules

    def drain(n):
        for _ in range(n):
            q = proj_fill if proj_fill else out_fill
            if not q:
                return
            g = q.popleft()
            try:
                next(g)
                q.append(g)
            except StopIteration:
                pass

    def force(q):
        while q:
            g = q.popleft()
            for _ in g:
                pass

    for cc in range(NCC):
        load_x(cc)
    for jt in range(4):
        for _ in qk_proj(jt, 0):
            pass
    for tt in range(4):
        for _ in v_proj(tt):
            pass
    for tb in range(nqb):
        force(proj_fill)  # attention(tb) needs block tb's projections traced
        if tb + 1 < nqb:
            for jt in range(4):
                proj_fill.append(qk_proj(jt, tb + 1))
            for tt in range((tb + 1) * 4, (tb + 1) * 4 + 4):
                proj_fill.append(v_proj(tt))
        for _ in attn_sx(tb, 0):
            drain(1)
        sx1 = attn_sx(tb, 1)
        proj_fill.appendleft(sx1)  # weave p1's S^T/exp into p0's V phase
        for _ in attn_v(tb, 0):
            drain(2)
        if sx1 in proj_fill:
            proj_fill.remove(sx1)
            for _ in sx1:
                drain(1)
        for _ in attn_v(tb, 1):
            drain(1)
        for tt in range(tb * 4, tb * 4 + 4):
            for cb in range(2):
                out_fill.append(out_proj(tt, cb))
    force(proj_fill)
    force(out_fill)
    ctx.close()


def host_inputs(x, w_attn, w_proj, t=T):
    """Build the 8 per-core input maps from full inputs."""
    xTs = [np.ascontiguousarray(x[b, :t].T).astype(BF) for b in range(B)]
    inv = 1.0 / (10000.0 ** (np.arange(0, HD, 2, dtype=np.float32) / HD))
    fr = np.outer(np.arange(t, dtype=np.float32), inv)     # [t, 32]
    emb = np.concatenate([fr, fr], 1)                      # [t, 64]
    cos = np.cos(emb).T.astype(np.float32)                 # [64, t]
    sin = np.sin(emb).T.astype(np.float32)
    sin_s = sin.copy()
    sin_s[:32] *= -1.0
    cosT2 = np.tile(cos, (2, 1)).astype(BF)
    sinT2 = np.tile(sin_s, (2, 1)).astype(BF)

    # within-tile causal mask: keep iff f >= p (f = query col, p = key row)
    f = np.arange(KT)[None, :]
    pp = np.arange(128)[:, None]
    mask = (f >= pp).astype(BF)                            # [128, KT]
    ident = np.eye(128, dtype=BF)

    in_maps = []
    for c in range(8):
        b, g = c // 4, c % 4
        wq = w_attn[g * GD:(g + 1) * GD]
        wk = w_attn[C + g * GD:C + (g + 1) * GD]
        wv = w_attn[2 * C + g * GD:2 * C + (g + 1) * GD]
        wqkT = np.ascontiguousarray(
            np.concatenate([wq, wk], 0).T).astype(BF)
        wvT = np.zeros((C, HG * (HD + 1)), BF)
        for h in range(HG):
            wvT[:, h * (HD + 1):h * (HD + 1) + HD] = \
                wv[h * HD:(h + 1) * HD].T.astype(BF)
        wpT = np.ascontiguousarray(
            w_proj[:, g * GD:(g + 1) * GD].T).astype(BF)
        in_maps.append({"xT": xTs[b], "wqkT": wqkT, "wvT": wvT,
                        "wpT": wpT, "cosT": cosT2, "sinT": sinT2,
                        "mask": mask, "ident": ident})
    return in_maps


_cache = {}


def kernel(x, w_attn, w_proj):
    x = np.asarray(x, dtype=np.float32)
    w_attn = np.asarray(w_attn, dtype=np.float32)
    w_proj = np.asarray(w_proj, dtype=np.float32)
    if "nc" not in _cache:
        _cache["nc"] = build_kernel()
    nc = _cache["nc"]
    in_maps = host_inputs(x, w_attn, w_proj)
    res = run_bass_kernel_spmd(nc, in_maps, list(range(8)))
    out = np.zeros((B, T, C), dtype=np.float32)
    for c in range(8):
        out[c // 4] += res.results[c]["out"].astype(np.float32)
    return out
